# revision 9
# baseline (speedup 1.0000x reference)
"""GCN 2-layer message passing on 8 Trainium2 NeuronCores (Bass/Tile).

Algorithm (matches PyG GCNConv reference):
    deg  = 1 + scatter_add(ew, dst);  dinv = deg^-1/2
    h1   = relu(Anorm @ (x @ W1) + b1)
    out  = log_softmax((Anorm @ h1) @ W2 + b2)
with Anorm = D^-1/2 (A + I) D^-1/2 (weighted self-loops of weight 1).

Distribution: nodes are sharded 12544/core across 8 cores (1D graph
partition by destination). Each core computes its shard of h = x @ W1,
all-gathers the (tiny, 16-wide) transposed feature table, then processes
its incoming edges:
  - edges are bucketed by source shard onto the 8 GpSimd Q7 cores; each
    Q7 core holds one source shard's features as a per-partition table
    (layout [16 feats, nodes]) and gathers the per-edge source features
    with ap_gather (indices are host-precomputed build-time constants)
  - messages are scaled by the per-edge weight ew*dinv[dst] (DVE),
    prefix-summed along the edge axis (tensor_tensor_scan), and segment
    sums are extracted as differences of the prefix at host-precomputed
    destination boundaries (second ap_gather + subtract)
  - per-source-shard partials are reduced across the 8 partition groups
    with a single 0/1-selector matmul on the TensorEngine; the self-loop
    term dinv^2*h is added in canonical layout.
Both layers aggregate in the 16-wide space (layer 2 aggregates before
applying W2, exploiting A @ (H W) = (A @ H) W), so all gathers move
16-float rows only.
"""

import numpy as np
import ml_dtypes

BF16 = ml_dtypes.bfloat16

# ---------------------------------------------------------------- dims
N_NODES = 100000
N_FEAT = 512
HIDDEN = 16
N_CLASSES = 64
N_CORES = 8

FULL_DIMS = dict(
    n_nodes=N_NODES,
    shard=12544,            # nodes per core (8*12544 = 100352 >= 100000)
    pieces=16,              # dest-range pieces per (core, group)
    lp=3584,                # padded edge slots per (group, piece), mult of 16
    ntile=448,              # node cols per PSUM tile (28 tiles of 448)
    n_feat=N_FEAT,
    hidden=HIDDEN,
    n_classes=N_CLASSES,
)

GROUPS = 8  # q7 cores / partition groups


def _dims_derived(dims):
    d = dict(dims)
    d["dpiece"] = d["shard"] // d["pieces"] // GROUPS * GROUPS
    assert d["shard"] % d["pieces"] == 0
    d["dpiece"] = d["shard"] // d["pieces"]
    nb = d["dpiece"] + 1
    d["nb"] = ((nb + 15) // 16) * 16  # padded boundary-gather count
    assert d["shard"] % d["ntile"] == 0
    d["ntiles"] = d["shard"] // d["ntile"]
    assert d["n_feat"] % 128 == 0
    d["fchunks"] = d["n_feat"] // 128
    assert d["lp"] % 16 == 0
    d["n_total"] = d["shard"] * N_CORES
    return d


# ------------------------------------------------------- host preprocess
def _prep_inputs(x, edge_index, edge_weight, W1, b1, W2, b2, dims):
    """Build the per-core DRAM input arrays (pure index/layout work +
    the degree normalization, all O(E) numpy)."""
    d = _dims_derived(dims)
    SH, PIECES, LP, NB, DP = d["shard"], d["pieces"], d["lp"], d["nb"], d["dpiece"]
    NTOT = d["n_total"]
    H = d["hidden"]
    n = x.shape[0]

    src = np.asarray(edge_index[0], dtype=np.int64)
    dst = np.asarray(edge_index[1], dtype=np.int64)
    ew = np.asarray(edge_weight, dtype=np.float32)

    deg = np.bincount(dst, weights=ew.astype(np.float64), minlength=NTOT)
    deg = deg.astype(np.float32) + 1.0  # self-loop weight 1; padded nodes -> 1
    dinv = (1.0 / np.sqrt(deg)).astype(np.float32)

    wq = (ew * dinv[dst]).astype(np.float32)  # dinv[src] folded into table

    nc_id = dst // SH
    grp = src // SH
    dl = dst % SH
    sl = (src % SH).astype(np.int16)
    piece = dl // DP

    bucket = ((nc_id * GROUPS + grp) * PIECES + piece).astype(np.int64)
    order = np.argsort(bucket * SH + dl, kind="stable")
    b_sorted = bucket[order]
    counts = np.bincount(bucket, minlength=N_CORES * GROUPS * PIECES)
    assert counts.max() <= LP - 1, f"piece overflow: {counts.max()} > {LP - 1}"
    offs = np.zeros_like(counts)
    np.cumsum(counts[:-1], out=offs[1:])
    rank = np.arange(len(order)) - offs[b_sorted]

    idx_all = np.zeros((N_CORES, GROUPS, PIECES, LP), dtype=np.int16)
    ew_all = np.zeros((N_CORES, GROUPS, PIECES, LP), dtype=np.float32)
    flat_slot = b_sorted * LP + 1 + rank  # slot 0 is the zero dummy
    idx_all.reshape(-1)[flat_slot] = sl[order]
    ew_all.reshape(-1)[flat_slot] = wq[order]

    # boundary prefix positions: cum[d] = 1-based slot of last edge of dest d
    cnt_d = np.bincount((nc_id * GROUPS + grp) * SH + dl,
                        minlength=N_CORES * GROUPS * SH)
    cnt_d = cnt_d.reshape(N_CORES, GROUPS, PIECES, DP)
    cum = cnt_d.cumsum(axis=3).astype(np.int16)
    bidx_all = np.zeros((N_CORES, GROUPS, PIECES, NB), dtype=np.int16)
    bidx_all[:, :, :, 1:DP + 1] = cum
    bidx_all[:, :, :, DP + 1:] = cum[:, :, :, -1:]

    def wrap16(a):
        # [..., L] -> [..., 16, L//16]: flat pos t -> (t % 16, t // 16)
        L = a.shape[-1]
        return a.reshape(*a.shape[:-1], L // 16, 16).swapaxes(-1, -2)

    # [NC, 128, PIECES, L/16]
    idx_w = wrap16(idx_all).transpose(0, 1, 3, 2, 4).reshape(
        N_CORES, 128, PIECES, LP // 16).astype(np.int16)
    bidx_w = wrap16(bidx_all).transpose(0, 1, 3, 2, 4).reshape(
        N_CORES, 128, PIECES, NB // 16).astype(np.int16)
    # ew replicated x16 within each group: [NC, 128, PIECES, LP]
    ew_w = np.repeat(ew_all[:, :, None, :, :], 16, axis=2).reshape(
        N_CORES, 128, PIECES, LP).astype(BF16)

    # x transposed + padded: xT_w[i][p, c, n] = x[SH*i + n, 128*c + p]
    xpad = np.zeros((NTOT, d["n_feat"]), dtype=np.float32)
    xpad[:n] = np.asarray(x, dtype=np.float32)
    xT = xpad.reshape(N_CORES, SH, d["fchunks"], 128)
    xT_w = np.ascontiguousarray(xT.transpose(0, 3, 2, 1)).astype(BF16)

    dinvr = np.broadcast_to(
        dinv.reshape(N_CORES, 1, SH), (N_CORES, 128, SH)).astype(BF16)

    W1 = np.asarray(W1, dtype=np.float32)
    W1c = np.ascontiguousarray(
        W1.reshape(d["fchunks"], 128, H).transpose(1, 0, 2)
    ).reshape(128, d["fchunks"] * H).astype(BF16)

    S = np.zeros((128, H), dtype=np.float32)
    for g in range(GROUPS):
        S[g * 16:g * 16 + H, :] = np.eye(H, dtype=np.float32)
    S = S.astype(BF16)

    W2c = np.asarray(W2, dtype=np.float32).astype(BF16)
    b2t = np.broadcast_to(np.asarray(b2, dtype=np.float32),
                          (128, len(b2))).copy()
    b1c = np.asarray(b1, dtype=np.float32).reshape(H, 1).copy()

    in_maps = []
    for i in range(N_CORES):
        in_maps.append({
            "xT": np.ascontiguousarray(xT_w[i]),
            "idx": np.ascontiguousarray(idx_w[i]),
            "bidx": np.ascontiguousarray(bidx_w[i]),
            "ew": np.ascontiguousarray(ew_w[i]),
            "dinvr": np.ascontiguousarray(dinvr[i]),
            "W1c": W1c,
            "S": S,
            "W2b": W2c,
            "b2t": b2t,
            "b1c": b1c,
        })
    return in_maps


# ------------------------------------------------------- device program
def _build_program(dims):
    import sys
    if "/opt/trn_rl_repo" not in sys.path:
        sys.path.insert(0, "/opt/trn_rl_repo")
    from concourse import bass, bacc, tile, mybir

    d = _dims_derived(dims)
    SH, PIECES, LP, NB, DP = d["shard"], d["pieces"], d["lp"], d["nb"], d["dpiece"]
    NT, NTL = d["ntiles"], d["ntile"]
    H, C, FC = d["hidden"], d["n_classes"], d["fchunks"]
    f32, bf16, i16 = mybir.dt.float32, mybir.dt.bfloat16, mybir.dt.int16
    ADD, SUB, MULT = (mybir.AluOpType.add, mybir.AluOpType.subtract,
                      mybir.AluOpType.mult)
    AF = mybir.ActivationFunctionType

    nc = bacc.Bacc("TRN2", debug=False, num_devices=N_CORES)

    xT = nc.dram_tensor("xT", [128, FC, SH], bf16, kind="ExternalInput")
    idx = nc.dram_tensor("idx", [128, PIECES, LP // 16], i16, kind="ExternalInput")
    bidx = nc.dram_tensor("bidx", [128, PIECES, NB // 16], i16, kind="ExternalInput")
    ewt = nc.dram_tensor("ew", [128, PIECES, LP], bf16, kind="ExternalInput")
    dinvr = nc.dram_tensor("dinvr", [128, SH], bf16, kind="ExternalInput")
    W1c = nc.dram_tensor("W1c", [128, FC * H], bf16, kind="ExternalInput")
    S = nc.dram_tensor("S", [128, H], bf16, kind="ExternalInput")
    W2b = nc.dram_tensor("W2b", [H, C], bf16, kind="ExternalInput")
    b2t = nc.dram_tensor("b2t", [128, C], f32, kind="ExternalInput")
    b1c = nc.dram_tensor("b1c", [H, 1], f32, kind="ExternalInput")
    out = nc.dram_tensor("out", [SH, C], f32, kind="ExternalOutput")

    with tile.TileContext(nc) as tc:
        with (
            tc.tile_pool(name="const", bufs=1) as constp,
            tc.tile_pool(name="canon", bufs=1) as canonp,
            tc.tile_pool(name="table", bufs=1) as tablep,
            tc.tile_pool(name="acc", bufs=1) as accp,
            tc.tile_pool(name="edge", bufs=2) as edgep,
            tc.tile_pool(name="scan", bufs=1) as scanp,
            tc.tile_pool(name="mmx", bufs=2) as mmxp,
            tc.tile_pool(name="epi", bufs=2) as epip,
            tc.tile_pool(name="psum16", bufs=2, space="PSUM") as psum16p,
            tc.tile_pool(name="psumo", bufs=2, space="PSUM") as psumop,
            tc.tile_pool(name="dram", bufs=1, space="DRAM") as dramp,
        ):
            # ---- constants
            t_W1c = constp.tile([128, FC * H], bf16)
            nc.sync.dma_start(out=t_W1c[:], in_=xW(W1c))
            t_S = constp.tile([128, H], bf16)
            nc.sync.dma_start(out=t_S[:], in_=xW(S))
            t_W2b = constp.tile([H, C], bf16)
            nc.sync.dma_start(out=t_W2b[:], in_=xW(W2b))
            t_b2 = constp.tile([128, C], f32)
            nc.sync.dma_start(out=t_b2[:], in_=xW(b2t))
            t_b1 = constp.tile([H, 1], f32)
            nc.sync.dma_start(out=t_b1[:], in_=xW(b1c))

            # canonical [128, SH] bf16, 32-aligned partition bases:
            # agg2@0, T_can@32 (T2 reuses it), self@64
            canon = canonp.tile([128, SH], bf16)
            R_AGG2, R_T, R_SELF = 0, 32, 64
            R_T2 = R_T
            t_dinv = canonp.tile([128, SH], bf16, name="t_dinv")
            nc.sync.dma_start(out=t_dinv[:], in_=xW(dinvr))

            # zero data0 for the scans (free-dim broadcast)
            zeros = scanp.tile([128, 1], bf16, tag="zeros")
            nc.vector.memset(zeros[:], 0.0)

            # DRAM bounce buffers for the all-gather
            ag_in = [dramp.tile([H, SH], bf16, tag=f"agin{l}", name=f"agin{l}") for l in range(2)]
            ag_out = [dramp.tile([128, SH], bf16, tag=f"agout{l}",
                                 name=f"agout{l}", addr_space="Shared")
                      for l in range(2)]

            # ---- phase 1: h1 = x @ W1, scaled by dinv -> T_can (canonical)
            for t in range(NT):
                xtile = mmxp.tile([128, FC, NTL], bf16, tag="xtile")
                nc.sync.dma_start(out=xtile[:],
                                  in_=xW(xT)[:, :, t * NTL:(t + 1) * NTL])
                ps = psum16p.tile([H, NTL], f32, tag="ps16")
                for c in range(FC):
                    nc.tensor.matmul(ps[:], t_W1c[:, c * H:(c + 1) * H],
                                     xtile[:, c, :], start=(c == 0),
                                     stop=(c == FC - 1))
                nc.vector.tensor_tensor(
                    canon[R_T:R_T + H, t * NTL:(t + 1) * NTL], ps[:],
                    t_dinv[R_T:R_T + H, t * NTL:(t + 1) * NTL], MULT)

            table = tablep.tile([128, SH], f32)
            acc = accp.tile([128, SH], bf16)

            def all_gather_layer(l, src_rows):
                nc.sync.dma_start(out=ag_in[l][:], in_=canon[src_rows:src_rows + H, :])
                nc.gpsimd.collective_compute(
                    "AllGather", mybir.AluOpType.bypass,
                    replica_groups=[list(range(N_CORES))],
                    ins=[ag_in[l].opt()], outs=[ag_out[l].opt()])
                nc.gpsimd.dma_start(out=table[:], in_=ag_out[l][:])  # bf16->f32 cast
                # self term: dinv * T
                nc.vector.tensor_tensor(
                    canon[R_SELF:R_SELF + H, :], canon[src_rows:src_rows + H, :],
                    t_dinv[src_rows:src_rows + H, :], MULT)

            def edge_pipeline():
                for j in range(PIECES):
                    t_idx = edgep.tile([128, LP // 16], i16, tag="idx")
                    nc.sync.dma_start(out=t_idx[:], in_=xW(idx)[:, j, :])
                    t_ew = edgep.tile([128, LP], bf16, tag="ew", bufs=1)
                    nc.sync.dma_start(out=t_ew[:], in_=xW(ewt)[:, j, :])
                    t_bidx = edgep.tile([128, NB // 16], i16, tag="bidx")
                    nc.sync.dma_start(out=t_bidx[:], in_=xW(bidx)[:, j, :])

                    g = edgep.tile([128, LP], f32, tag="gather")
                    nc.gpsimd.ap_gather(g[:], table[:], t_idx[:],
                                        channels=128, num_elems=SH, d=1,
                                        num_idxs=LP)
                    nc.vector.tensor_tensor(g[:], g[:], t_ew[:], MULT)
                    P_ = scanp.tile([128, LP], f32, tag="P")
                    nc.vector.tensor_tensor_scan(
                        P_[:], zeros.broadcast_to([128, LP]), g[:], 0.0,
                        ADD, ADD)
                    G = edgep.tile([128, NB], f32, tag="G")
                    nc.gpsimd.ap_gather(G[:], P_[:], t_bidx[:],
                                        channels=128, num_elems=LP, d=1,
                                        num_idxs=NB)
                    nc.vector.tensor_tensor(acc[:, j * DP:(j + 1) * DP],
                                            G[:, 1:DP + 1], G[:, 0:DP], SUB)

            # ---- layer 1
            all_gather_layer(0, R_T)
            edge_pipeline()
            # reduce across groups + epilogue -> T2_can
            for t in range(NT):
                sl = slice(t * NTL, (t + 1) * NTL)
                ps = psum16p.tile([H, NTL], f32, tag="psr")
                nc.tensor.matmul(ps[:], t_S[:], acc[:, sl], start=True, stop=True)
                tmp = epip.tile([H, NTL], f32, tag="tmp")
                nc.vector.tensor_tensor(tmp[:], ps[:], canon[R_SELF:R_SELF + H, sl],
                                        ADD)
                rl = epip.tile([H, NTL], bf16, tag="relu")
                nc.scalar.activation(rl[:], tmp[:], AF.Relu, bias=t_b1[:])
                nc.vector.tensor_tensor(canon[R_T2:R_T2 + H, sl], rl[:],
                                        t_dinv[0:H, sl], MULT)

            # ---- layer 2
            all_gather_layer(1, R_T2)
            edge_pipeline()
            for t in range(NT):
                sl = slice(t * NTL, (t + 1) * NTL)
                ps = psum16p.tile([H, NTL], f32, tag="psr")
                nc.tensor.matmul(ps[:], t_S[:], acc[:, sl], start=True, stop=True)
                nc.vector.tensor_tensor(canon[R_AGG2:R_AGG2 + H, sl], ps[:],
                                        canon[R_SELF:R_SELF + H, sl], ADD)

            # ---- final matmul + log_softmax, batches of 8 node-tiles of 128
            n_node_tiles = SH // 128
            BATCH = 8
            for b0 in range(0, n_node_tiles, BATCH):
                nb_t = min(BATCH, n_node_tiles - b0)
                pso = psumop.tile([128, BATCH, C], f32, tag="pso")
                for k in range(nb_t):
                    col = (b0 + k) * 128
                    nc.tensor.matmul(pso[:, k, :],
                                     canon[R_AGG2:R_AGG2 + H, col:col + 128],
                                     t_W2b[:], start=True, stop=True)
                lg = epip.tile([128, BATCH, C], f32, tag="lg")
                nc.vector.tensor_tensor(
                    lg[:, :nb_t, :], pso[:, :nb_t, :],
                    t_b2[:].rearrange("p (o c) -> p o c", o=1).broadcast_to(
                        [128, nb_t, C]), ADD)
                ex = epip.tile([128, BATCH, C], f32, tag="ex")
                nc.scalar.activation(ex[:, :nb_t, :], lg[:, :nb_t, :], AF.Exp)
                ssum = epip.tile([128, BATCH], f32, tag="ssum")
                nc.vector.tensor_reduce(ssum[:, :nb_t], ex[:, :nb_t, :],
                                        mybir.AxisListType.X, ADD)
                lsum = epip.tile([128, BATCH], f32, tag="lsum")
                nc.scalar.activation(lsum[:, :nb_t], ssum[:, :nb_t], AF.Ln)
                ot = epip.tile([128, BATCH, C], f32, tag="ot")
                nc.vector.tensor_tensor(
                    ot[:, :nb_t, :], lg[:, :nb_t, :],
                    lsum[:, :nb_t].broadcast_to([128, nb_t, C]), SUB)
                # out rows: node-tile k -> out[(b0+k)*128 : +128, :]
                nc.sync.dma_start(
                    out=xW(out)[b0 * 128:(b0 + nb_t) * 128, :].rearrange(
                        "(k p) c -> p k c", p=128),
                    in_=ot[:, :nb_t, :])

    nc.compile()
    return nc


def xW(t):
    """DRAM tensor handle -> AP."""
    return t.ap()


_PROGRAM_CACHE = {}


def _get_program(dims):
    key = tuple(sorted(dims.items()))
    if key not in _PROGRAM_CACHE:
        _PROGRAM_CACHE[key] = _build_program(dims)
    return _PROGRAM_CACHE[key]


def _install_ntff_shim():
    """Register the NTFF profile hook that the agent image's antenv lacks."""
    import types, ctypes, contextlib
    so_path = "/opt/axon/libaxon_pjrt.so"
    try:
        import antenv
        from antenv.axon_hooks import get_axon_ntff_profile_hook  # noqa
        return True  # already present
    except ImportError:
        pass
    try:
        lib = ctypes.CDLL(so_path)
    except OSError:
        return False
    if not hasattr(lib, "axon_start_nrt_profile"):
        return False
    lib.axon_start_nrt_profile.argtypes = [ctypes.POINTER(ctypes.c_int64),
                                           ctypes.c_size_t]
    lib.axon_start_nrt_profile.restype = ctypes.c_int64
    lib.axon_stop_nrt_profile.argtypes = [ctypes.c_char_p]
    lib.axon_stop_nrt_profile.restype = ctypes.c_int64

    @contextlib.contextmanager
    def _hook(output_dir, device_ids):
        import jax
        jax.devices()
        if device_ids:
            ids = (ctypes.c_int64 * len(device_ids))(*device_ids)
            rc = lib.axon_start_nrt_profile(ids, len(device_ids))
        else:
            rc = lib.axon_start_nrt_profile(None, 0)
        if rc != 0:
            raise RuntimeError(f"axon_start_nrt_profile rc={rc}")
        try:
            yield
        finally:
            n = lib.axon_stop_nrt_profile(str(output_dir).encode())
            if n < 0:
                raise RuntimeError(f"axon_stop_nrt_profile rc={n}")

    import sys as _s, antenv
    mod = types.ModuleType("antenv.axon_hooks")
    mod.set_axon_ntff_profile_hook = lambda h: None
    mod.get_axon_ntff_profile_hook = lambda: _hook
    _s.modules["antenv.axon_hooks"] = mod
    antenv.axon_hooks = mod
    return True


def run(x, edge_index, edge_weight, W1, b1, W2, b2, trace=False):
    import sys
    if "/opt/trn_rl_repo" not in sys.path:
        sys.path.insert(0, "/opt/trn_rl_repo")
    from concourse.bass_utils import run_bass_kernel_spmd

    if trace:
        _install_ntff_shim()
    dims = FULL_DIMS
    in_maps = _prep_inputs(x, edge_index, edge_weight, W1, b1, W2, b2, dims)
    nc = _get_program(dims)
    res = run_bass_kernel_spmd(nc, in_maps, core_ids=list(range(N_CORES)),
                               trace=trace)
    outs = [res.results[i]["out"] for i in range(N_CORES)]
    full = np.concatenate(outs, axis=0)[:x.shape[0]].astype(np.float32)
    return full, res


def kernel(x, edge_index, edge_weight, W1, b1, W2, b2):
    full, _ = run(x, edge_index, edge_weight, W1, b1, W2, b2, trace=False)
    return full
